# revision 36
# baseline (speedup 1.0000x reference)
"""Trainium2 Bass kernel for nn_CPLinear (CP-decomposed QKV projection with RoPE).

Computes, for x:(2,4096,2048) and CP-factor weights:
    A_t = x @ W_A_t  (per-token head coefficients),  B_t = x @ W_B_t (shared bases)
    q = einsum('bshr,bsrd->bshd', A_q, rope(B_q)) / 12
    k = A_k * rope(B_k)   (rank-1)
    v = A_v * B_v         (rank-1)

Strategy (8 cores, data-parallel over the 8192 tokens, 1024 tokens/core):
  - All 6 projections fused into one [2048 x 2016] bf16 matmul (PE), with the
    1/12 scale and (h,r)->(r,h) reorder folded into W_A_q host-side.
  - x is uploaded as bf16 and loaded transposed via the DMA xbar transpose so
    the contraction dim lands on partitions with no on-chip transposes.
  - RoPE applied to B_q/B_k with bf16 tensor_tensor ops (cos/sin tables are
    host-precomputed per-token inputs, replicated x12 along r).
  - The per-token rank-12 contraction for q runs on the PE as a block-diagonal
    matmul: 8 tokens/matmul, K=96=(8 tokens x 12 r), M=128=(8 tokens x 16 h),
    N=128=d. Operands are built by partition-interleaving scatter DMAs.
  - k/v are per-partition-scalar broadcasts (DVE tensor_scalar / ACT activation).
  - Outputs are written bf16 and widened to fp32 on the host.
"""

import sys

for _p in ("/opt/trn_rl_repo",):
    if _p not in sys.path:
        sys.path.insert(0, _p)

import numpy as np
import ml_dtypes

BF16 = ml_dtypes.bfloat16

SH = 1024          # tokens per core
H = 2048           # hidden
KT = H // 128      # 16 k-tiles
NT = SH // 128     # 8 token tiles per core
NOUT = 2016        # fused projection output width
NH, HD, RQ = 16, 128, 12

_CACHE = {}


def make_nc():
    import concourse.bacc as bacc
    from concourse import mybir

    dt = mybir.dt

    nc = bacc.Bacc(
        "TRN2",
        target_bir_lowering=False,
        debug=False,
        enable_asserts=False,
        num_devices=8,
    )

    x_d = nc.dram_tensor("x", (H, SH), dt.bfloat16, kind="ExternalInput")  # pre-transposed host-side
    w_d = nc.dram_tensor("w", (KT, 128, NOUT), dt.bfloat16, kind="ExternalInput")
    cos_d = nc.dram_tensor("cosr", (SH, 64), dt.bfloat16, kind="ExternalInput")
    sin_d = nc.dram_tensor("sinr", (SH, 64), dt.bfloat16, kind="ExternalInput")
    q_d = nc.dram_tensor("q", (SH, NH, HD), dt.bfloat16, kind="ExternalOutput")
    k_d = nc.dram_tensor("k", (SH, NH * HD), dt.bfloat16, kind="ExternalOutput")
    v_d = nc.dram_tensor("v", (SH, NH * HD), dt.bfloat16, kind="ExternalOutput")
    return nc, (x_d, w_d, cos_d, sin_d, q_d, k_d, v_d)


def build_body(nc, tc, tensors):
    from contextlib import ExitStack

    from concourse import mybir

    dt = mybir.dt
    x_d, w_d, cos_d, sin_d, q_d, k_d, v_d = tensors

    with ExitStack() as ctx:
        P = ctx.enter_context
        const_pool = P(tc.tile_pool(name="const", bufs=1))
        w_sb = const_pool.tile([128, KT * NOUT], dt.bfloat16, tag="w_sb")
        cos_sb = const_pool.tile([128, NT * 64], dt.bfloat16, tag="cos_sb")
        sin_sb = const_pool.tile([128, NT * 64], dt.bfloat16, tag="sin_sb")
        xT = const_pool.tile([128, KT * SH], dt.bfloat16, tag="xT")
        # ping-pong block-diagonal lhsT holders for the q contraction
        lhs_bufs = [
            const_pool.tile(
                [128, 2048], dt.bfloat16, tag=f"lhs{i}", name=f"lhs{i}"
            )
            for i in range(3)
        ]

        # constant loads — quarter-merged DMAs (pipelines the first matmuls
        # while keeping the HWDGE instruction count low); SBUF dims stay
        # partition-first, the reorder lives on the DRAM side of the AP.
        w_v = w_sb[:].rearrange("p (k n) -> p k n", k=KT)
        wd_v = w_d[:].rearrange("k p n -> p k n")
        x_v = xT[:].rearrange("p (k t) -> p k t", k=KT)
        xd_v = x_d[:].rearrange("(k p) t -> p k t", p=128)
        nc.gpsimd.dma_start(
            out=cos_sb[:].rearrange("p (t n) -> p t n", t=NT),
            in_=cos_d[:].rearrange("(t p) n -> p t n", p=128),
        )
        nc.gpsimd.dma_start(
            out=sin_sb[:].rearrange("p (t n) -> p t n", t=NT),
            in_=sin_d[:].rearrange("(t p) n -> p t n", p=128),
        )
        for qtr in range(4):
            sl = slice(qtr * 4, (qtr + 1) * 4)
            nc.scalar.dma_start(out=w_v[:, sl], in_=wd_v[:, sl])
            nc.sync.dma_start(out=x_v[:, sl], in_=xd_v[:, sl])
        # bdr ping-pong buffers: persistent + memset once (shields the sim's
        # conservative write-coverage tracking for the merged readback AP)
        bdr_bufs = [
            const_pool.tile(
                [128, 2048], dt.bfloat16, tag=f"bdr{i}", name=f"bdr{i}"
            )
            for i in range(3)
        ]
        for tl in lhs_bufs + bdr_bufs:
            nc.gpsimd.memset(tl[:], 0.0)

        psa_pool = P(tc.tile_pool(name="psa", bufs=2, space="PSUM"))
        psb_pool = P(tc.tile_pool(name="psb", bufs=2, space="PSUM"))
        bq_pool = P(tc.tile_pool(name="bq", bufs=2))
        bqr_pool = P(tc.tile_pool(name="bqr", bufs=2))
        tmp_pool = P(tc.tile_pool(name="tmp", bufs=2))
        small_pool = P(tc.tile_pool(name="small", bufs=4))
        out_pool = P(tc.tile_pool(name="outs", bufs=2))
        dram_pool = P(tc.tile_pool(name="scr", bufs=2, space="DRAM"))

        from concourse.ap import AP

        # per-tile state carried across the software-pipeline stages
        state = {}

        def s1_chunk(it, ps_a, ps_b, k_lo, k_hi):
            t0 = it * 128
            for kk in range(k_lo, k_hi):
                lh = xT[:, kk * SH + t0 : kk * SH + t0 + 128]
                wb = kk * NOUT
                st = kk == 0
                sp = kk == KT - 1
                nc.tensor.matmul(
                    ps_a[:, 0:480], lh, w_sb[:, wb : wb + 480], start=st, stop=sp
                )
                for c in range(3):
                    nc.tensor.matmul(
                        ps_b[:, c * 512 : (c + 1) * 512],
                        lh,
                        w_sb[:, wb + 480 + c * 512 : wb + 480 + (c + 1) * 512],
                        start=st,
                        stop=sp,
                    )

        def produce(it, ps=None):
            """step-1 projection, evictions, RoPE, scatter for tile `it`."""
            t0 = it * 128
            lhs = lhs_bufs[it % 3]
            bdr = bdr_bufs[it % 3]

            if ps is None:
                ps_a = psa_pool.tile([128, 512], dt.float32, tag="ps_a")
                ps_b = psb_pool.tile([128, 1536], dt.float32, tag="ps_b")
                s1_chunk(it, ps_a, ps_b, 0, KT)
            else:
                ps_a, ps_b = ps

            # ---- PSUM evictions (ACT) ----
            ak_sb = small_pool.tile([128, 16], dt.bfloat16, tag="ak_sb")
            av_sb = small_pool.tile([128, 16], dt.bfloat16, tag="av_sb")
            bk_sb = small_pool.tile([128, 128], dt.bfloat16, tag="bk_sb")
            bkr_sb = small_pool.tile([128, 128], dt.bfloat16, tag="bkr_sb")
            bv_sb = small_pool.tile([128, 128], dt.bfloat16, tag="bv_sb")
            bq_sb = bq_pool.tile([128, 1536], dt.bfloat16, tag="bq_sb")
            # bqr holds roped B_q (cols 0:1536) and A' (cols 1536:1728) so the
            # DRAM bounce is a single DMA
            bqr = bqr_pool.tile([128, 1728], dt.bfloat16, tag="bqr_t")
            nc.scalar.copy(bqr[:, 1536:1728], ps_a[:, 0:192])
            nc.scalar.copy(ak_sb[:], ps_a[:, 192:208])
            nc.scalar.copy(av_sb[:], ps_a[:, 208:224])
            nc.scalar.copy(bk_sb[:], ps_a[:, 224:352])
            nc.scalar.copy(bv_sb[:], ps_a[:, 352:480])
            nc.scalar.copy(bq_sb[:], ps_b[:, 0:1536])

            # ---- RoPE on B_q (DVE, bf16) ----
            t_a = tmp_pool.tile([128, 768], dt.bfloat16, tag="t_a")
            t_b = tmp_pool.tile([128, 768], dt.bfloat16, tag="t_b")
            bqv = bq_sb[:].rearrange("p (r two d) -> p r two d", r=RQ, two=2)
            bqrv = bqr[:, 0:1536].rearrange(
                "p (r two d) -> p r two d", r=RQ, two=2
            )
            cos_t = (
                cos_sb[:, it * 64 : (it + 1) * 64]
                .unsqueeze(1)
                .broadcast_to([128, RQ, 64])
            )
            sin_t = (
                sin_sb[:, it * 64 : (it + 1) * 64]
                .unsqueeze(1)
                .broadcast_to([128, RQ, 64])
            )
            tav = t_a[:].rearrange("p (r d) -> p r d", r=RQ)
            tbv = t_b[:].rearrange("p (r d) -> p r d", r=RQ)
            p_lo = bqv[:, :, 0]
            p_hi = bqv[:, :, 1]
            nc.vector.tensor_mul(tav, p_lo, cos_t)
            nc.vector.tensor_mul(tbv, p_hi, sin_t)
            nc.vector.tensor_sub(bqrv[:, :, 0], tav, tbv)
            nc.vector.tensor_mul(tav, p_hi, cos_t)
            nc.vector.tensor_mul(tbv, p_lo, sin_t)
            nc.vector.tensor_add(bqrv[:, :, 1], tav, tbv)

            # ---- RoPE on B_k (DVE, bf16) ----
            tk_a = small_pool.tile([128, 64], dt.bfloat16, tag="tk_a")
            tk_b = small_pool.tile([128, 64], dt.bfloat16, tag="tk_b")
            bkv = bk_sb[:].rearrange("p (two d) -> p two d", two=2)
            bkrv = bkr_sb[:].rearrange("p (two d) -> p two d", two=2)
            cos_k = cos_sb[:, it * 64 : (it + 1) * 64]
            sin_k = sin_sb[:, it * 64 : (it + 1) * 64]
            nc.vector.tensor_mul(tk_a[:], bkv[:, 0], cos_k)
            nc.vector.tensor_mul(tk_b[:], bkv[:, 1], sin_k)
            nc.vector.tensor_sub(bkrv[:, 0], tk_a[:], tk_b[:])
            nc.vector.tensor_mul(tk_a[:], bkv[:, 1], cos_k)
            nc.vector.tensor_mul(tk_b[:], bkv[:, 0], sin_k)
            nc.vector.tensor_add(bkrv[:, 1], tk_a[:], tk_b[:])

            # ---- scatter A', roped B_q into block-diagonal layout ----
            # Bounce through DRAM (partition-interleaves must keep the SBUF
            # side partition-leading); read back with ONE DMA per operand.
            scr = dram_pool.tile([128, 1728], dt.bfloat16, tag="scr_b")
            nc.sync.dma_start(out=scr[:], in_=bqr[:])

            # ---- k, v (DVE tensor_scalar) + outputs: independent of the
            # scatter chain, so emit them in the produce stage ----
            ksb = out_pool.tile([128, 2048], dt.bfloat16, tag="ksb")
            vsb = out_pool.tile([128, 2048], dt.bfloat16, tag="vsb")
            nc.vector.tensor_mul(
                ksb[:].rearrange("p (h d) -> p h d", h=NH),
                bkr_sb[:].unsqueeze(1).broadcast_to([128, NH, 128]),
                ak_sb[:].unsqueeze(2).broadcast_to([128, NH, 128]),
            )
            nc.vector.tensor_mul(
                vsb[:].rearrange("p (h d) -> p h d", h=NH),
                bv_sb[:].unsqueeze(1).broadcast_to([128, NH, 128]),
                av_sb[:].unsqueeze(2).broadcast_to([128, NH, 128]),
            )
            nc.sync.dma_start(out=k_d[t0 : t0 + 128, :], in_=ksb[:])
            nc.scalar.dma_start(out=v_d[t0 : t0 + 128, :], in_=vsb[:])
            sa_v = scr[:, 1536:1728].rearrange(
                "(g t) (r h) -> t r g h", t=8, r=RQ
            )
            sb_v = scr[:, 0:1536].rearrange("(g t) (r d) -> t r g d", t=8, r=RQ)
            l_v = lhs[0:96, :].rearrange("(t r) (g c) -> t r g c", t=8, g=16)
            d_v = bdr[0:96, :].rearrange("(t r) (g d) -> t r g d", t=8, g=16)
            for t in range(8):
                nc.gpsimd.dma_start(
                    out=l_v[t][:, :, t * 16 : (t + 1) * 16], in_=sa_v[t]
                )
                eng = nc.sync if t % 2 == 0 else nc.scalar
                eng.dma_start(out=d_v[t], in_=sb_v[t])

            state[it] = (lhs, bdr)

        def consume(it):
            """q contraction + k/v broadcast + output DMAs for tile `it`."""
            t0 = it * 128
            lhs, bdr = state.pop(it)

            # ---- q: block-diagonal matmuls (PE) + PSUM evict (DVE) ----
            qsb = out_pool.tile([128, 2048], dt.bfloat16, tag="qsb")
            for gq in range(4):
                qp = psa_pool.tile(
                    [128, 512], dt.float32, tag="ps_a", name=f"qp{it}_{gq}"
                )
                for j in range(4):
                    g = gq * 4 + j
                    nc.tensor.matmul(
                        qp[:, j * 128 : (j + 1) * 128],
                        lhs[0:96, g * 128 : (g + 1) * 128],
                        bdr[0:96, g * 128 : (g + 1) * 128],
                        start=True,
                        stop=True,
                    )
                nc.scalar.copy(qsb[:, gq * 512 : (gq + 1) * 512], qp[:])

            # ---- q output ----
            nc.scalar.dma_start(
                out=q_d[t0 : t0 + 128].rearrange("(g t) h d -> (t h) g d", g=16),
                in_=qsb[:].rearrange("p (g d) -> p g d", g=16),
            )

        # Tiles 0/1: interleave their step-1 matmuls at k-quarter granularity
        # so the PE rides the input stream instead of stalling on W quarters.
        ps0 = (
            psa_pool.tile([128, 512], dt.float32, tag="ps_a", name="ps_a0"),
            psb_pool.tile([128, 1536], dt.float32, tag="ps_b", name="ps_b0"),
        )
        ps1 = (
            psa_pool.tile([128, 512], dt.float32, tag="ps_a", name="ps_a1"),
            psb_pool.tile([128, 1536], dt.float32, tag="ps_b", name="ps_b1"),
        )
        for qtr in range(4):
            s1_chunk(0, *ps0, qtr * 4, (qtr + 1) * 4)
            s1_chunk(1, *ps1, qtr * 4, (qtr + 1) * 4)
        produce(0, ps=ps0)
        produce(1, ps=ps1)
        # 2-deep software pipeline: BD matmuls of tile i issue after step-1
        # of tile i+2, so the scatter chain latency hides behind PE work.
        for it in range(2, NT + 2):
            if it < NT:
                produce(it)
            if it >= 2:
                consume(it - 2)


def build_program():
    import concourse.tile as tile

    nc, tensors = make_nc()
    with tile.TileContext(nc) as tc:
        build_body(nc, tc, tensors)
    nc.compile()
    return nc


def _get_program():
    if "nc" not in _CACHE:
        _CACHE["nc"] = build_program()
    return _CACHE["nc"]


def make_in_maps(x, W_A_q, W_B_q, W_A_k, W_B_k, W_A_v, W_B_v):
    """Shard + preprocess full inputs into per-core input maps."""
    x = np.asarray(x)
    B, S, Hh = x.shape
    x2 = np.ascontiguousarray(x.reshape(B * S, Hh))

    # fold the 1/RQ scale and the (h,r)->(r,h) column reorder into W_A_q
    WAq = np.asarray(W_A_q).reshape(Hh, NH, RQ).transpose(0, 2, 1).reshape(
        Hh, NH * RQ
    ) / np.float32(RQ)
    Wall = np.concatenate(
        [
            WAq,
            np.asarray(W_A_k),
            np.asarray(W_A_v),
            np.asarray(W_B_k),
            np.asarray(W_B_v),
            np.asarray(W_B_q),
        ],
        axis=1,
    )
    assert Wall.shape == (Hh, NOUT)
    Wt = np.ascontiguousarray(Wall.reshape(KT, 128, NOUT)).astype(BF16)

    inv = 1.0 / (10000.0 ** (np.arange(0, HD, 2, dtype=np.float32) / HD))
    ang = np.arange(S, dtype=np.float32)[:, None] * inv[None, :]
    cos_rep = np.ascontiguousarray(np.cos(ang)).astype(BF16)
    sin_rep = np.ascontiguousarray(np.sin(ang)).astype(BF16)

    in_maps = []
    for i in range(8):
        tok0 = i * SH
        pos = np.arange(tok0, tok0 + SH) % S
        in_maps.append(
            {
                # pre-transposed (hidden, tokens) so on-chip loads are plain
                "x": np.ascontiguousarray(x2[tok0 : tok0 + SH].T).astype(BF16),
                "w": Wt,
                "cosr": np.ascontiguousarray(cos_rep[pos]),
                "sinr": np.ascontiguousarray(sin_rep[pos]),
            }
        )
    return in_maps, (B, S)


def assemble_outputs(results, B, S):
    q = np.concatenate(
        [results[i]["q"].astype(np.float32) for i in range(8)], axis=0
    ).reshape(B, S, NH, HD)
    k = np.concatenate(
        [results[i]["k"].astype(np.float32) for i in range(8)], axis=0
    ).reshape(B, S, NH, HD)
    v = np.concatenate(
        [results[i]["v"].astype(np.float32) for i in range(8)], axis=0
    ).reshape(B, S, NH, HD)
    return q, k, v


def kernel(x, W_A_q, W_B_q, W_A_k, W_B_k, W_A_v, W_B_v):
    from concourse.bass_utils import run_bass_kernel_spmd

    nc = _get_program()
    in_maps, (B, S) = make_in_maps(x, W_A_q, W_B_q, W_A_k, W_B_k, W_A_v, W_B_v)
    res = run_bass_kernel_spmd(nc, in_maps, list(range(8))).results
    return assemble_outputs(res, B, S)
